# revision 55
# baseline (speedup 1.0000x reference)
"""GroupedQueryAttention Trainium2 Bass kernel (8 NeuronCores, SPMD).

Reference quirk exploited: K/V are tiled R=4x along the group axis and
attention runs over the full concatenated 2048-dim. Mathematically this
collapses:
  scores = Q . tile(K)  ==  (sum of Q's four 512-chunks) . K      (512-dim)
  Z      = attn . tile(V)  -> tiled copies of  attn . V           (512-dim)
  out    = Z @ proj     ==  (attn . V) @ (sum of proj's 4 row-blocks)
So the whole module reduces to a single 512-dim attention:
  Qc = x @ WQc.T + bQc   (WQc = sum of WQ row-blocks)
  K  = x @ WK.T + bK ; V = x @ WV.T        (bV folded: see below)
  S  = Qc K^T (causal), softmax, /sqrt(128)
  y  = (softmax(S)/sqrt(128) V) @ projc    (projc = sum of proj row-blocks)
Because softmax rows sum to exactly 1, attn rows sum to 1/sqrt(128), so
attn @ (V + 1 bV^T) = attn @ V + bV/sqrt(128): the V bias becomes a
constant per-channel offset applied on the Z^T PSUM->SBUF copy instead
of rank-1 matmuls.

Sharding: 8 cores = 4 batches x 2 (interleaved 128-row blocks). Core with
pairpos q of batch b owns rows {256g+128q .. 256g+128q+127, g=0..7}. Causal
key extent per block rounds up to 256-multiples, which makes the tile counts
[1,2,3,4,5,6,7,8] identical on both cores of a pair -> a single SPMD program.
Per-core row selection / causal masks are host-prepared input tensors.

All matmul inputs are fp16 (full-speed PE mode, fp32 PSUM accumulation;
softmax statistics in fp32) — halves DMA traffic and LDWEIGHTS bandwidth
vs f32r at identical PE cycles/row. Host pre-arranges every tensor so
each device DMA is a single contiguous-per-partition descriptor; early
DMAs are chunk-staggered so the first matmul issues ~9us in.
Attention row-blocks are software-pipelined two deep; per iteration the
PE stream is  E^T/Z(cur) -> S(cur+2) -> O(prev) -> Z^T(cur)  so exp()
(scalar) and the zn normalize (DVE) always have PE work to hide behind,
and per-512-chunk output DMAs overlap compute. PSUM is managed as
explicit per-bank tag rings (S/O: 3, Z: 2, transposes: 3) so each tile
waits only on its own bank's previous user, never on a pool barrier.
"""

import numpy as np

import concourse.bacc as bacc
import concourse.mybir as mybir
from concourse.tile import TileContext
from concourse.bass_utils import run_bass_kernel_spmd

B, T, D = 4, 2048, 2048
HD = 512                 # collapsed head dim
NCORES = 8
RB = 8                   # 128-row blocks per core
KT_TILES = [1, 2, 3, 4, 5, 6, 7, 8]   # causal 256-wide key tiles per block
DCH = D // 128           # 16 contraction chunks
dt = mybir.dt
NEG = -60000.0           # fp16-safe mask value


def build_kernel():
    nc = bacc.Bacc(None, target_bir_lowering=False)

    # host-prepared layouts (see host_prep):
    #   XKV [128, 4, 16, 512] : x^T key-cols   (partition, key-group, c, key)
    #   XQ  [128, 2, 16, 512] : x^T query-cols (partition, row-group, c, row)
    #   WK/WV/WQ [128, 16, 512]: W^T           (partition, c, hd)
    #   PRJ [128, 4, 2048]    : projc          (partition, h, d)
    #   MSK [128, 8, 256]     : causal mask    (partition=row, block, key)
    XKV_d = nc.dram_tensor("XKV", [128, 4, DCH, 512], dt.float16, kind="ExternalInput")
    XQ_d = nc.dram_tensor("XQ", [128, 2, DCH, 512], dt.float16, kind="ExternalInput")
    WK_d = nc.dram_tensor("WK", [128, DCH, HD], dt.float16, kind="ExternalInput")
    WV_d = nc.dram_tensor("WV", [128, DCH, HD], dt.float16, kind="ExternalInput")
    WQ_d = nc.dram_tensor("WQ", [128, DCH, HD], dt.float16, kind="ExternalInput")
    PRJ_d = nc.dram_tensor("PRJ", [128, 4, D], dt.float16, kind="ExternalInput")
    bK_d = nc.dram_tensor("bK", [HD, 1], dt.float32, kind="ExternalInput")
    bQ_d = nc.dram_tensor("bQ", [HD, 1], dt.float32, kind="ExternalInput")
    bVs_d = nc.dram_tensor("bVs", [HD, 1], dt.float32, kind="ExternalInput")
    MSK_d = nc.dram_tensor("MSK", [128, RB, 256], dt.float16, kind="ExternalInput")
    IDT_d = nc.dram_tensor("IDT", [128, 128], dt.float16, kind="ExternalInput")
    OUT_d = nc.dram_tensor("out", [1024, D], dt.float32, kind="ExternalOutput")

    Act = mybir.ActivationFunctionType
    Ax = mybir.AxisListType

    with TileContext(nc) as tc:
        with tc.tile_pool(name="persist", bufs=1) as pp, \
             tc.tile_pool(name="xstream", bufs=2) as xp, \
             tc.tile_pool(name="scratch", bufs=1) as ap:
            # ---- persistent tiles ------------------------------------------
            wk = pp.tile([128, DCH * HD], dt.float16, tag="wk")
            xg = [xp.tile([128, DCH * 512], dt.float16, tag="xg", name=f"xg{rt}")
                  for rt in range(4)]
            # first-needed-first DMA order: single chunks unblock matmul 0 fast
            wv = pp.tile([128, DCH * HD], dt.float16, tag="wv")
            # stagger arrivals to match per-chunk consumption at the PE
            for lo, hi in ((0, 1), (1, 2), (2, 3), (3, 4), (4, 8), (8, 12),
                           (12, 16)):
                nc.sync.dma_start(out=wk[:, HD * lo:HD * hi], in_=WK_d[:, lo:hi])
                nc.sync.dma_start(out=xg[0][:, 512 * lo:512 * hi],
                                  in_=XKV_d[:, 0, lo:hi])
                nc.sync.dma_start(out=wv[:, HD * lo:HD * hi], in_=WV_d[:, lo:hi])
            KT_sb = [pp.tile([128, T], dt.float16, tag=f"KT{h}", name=f"KT{h}")
                     for h in range(4)]
            V_sb = [pp.tile([128, HD], dt.float16, tag=f"V{k}", name=f"V{k}")
                    for k in range(16)]
            QT_sb = [pp.tile([128, 1024], dt.float16, tag=f"QT{h}", name=f"QT{h}")
                     for h in range(4)]
            ZT_sb = [pp.tile([128, 1024], dt.float16, tag=f"ZT{h}", name=f"ZT{h}")
                     for h in range(4)]
            ident = pp.tile([128, 128], dt.float16, tag="ident")
            nc.sync.dma_start(out=ident[:], in_=IDT_d[:])
            bias_sb = {}
            for nm, src in (("bK", bK_d), ("bQ", bQ_d), ("bV", bVs_d)):
                for h in range(4):
                    t = pp.tile([128, 1], dt.float32, tag=f"{nm}{h}", name=f"{nm}{h}")
                    nc.sync.dma_start(out=t[:], in_=src[128 * h:128 * h + 128, :])
                    bias_sb[nm, h] = t

            # ---- stage P: K/V projections ----------------------------------
            # Per 512-key group: 4 PSUM banks accumulate K^T hd-tiles
            # ([hd, keys]) and 4 banks accumulate V key-blocks in [keys, hd]
            # layout (x chunk as lhsT).  K bias lands in the PSUM->SBUF copy.
            ps_cm = tc.tile_pool(name="psP", bufs=1, space="PSUM")
            ps = ps_cm.__enter__()
            for rt in range(4):
                if rt + 1 < 4:   # prefetch next x key-group
                    nc.sync.dma_start(out=xg[rt + 1][:], in_=XKV_d[:, rt + 1])
                if rt == 2:      # then queue the rest of the streamed inputs
                    xq = [xp.tile([128, DCH * 512], dt.float16, tag="xg",
                                  name=f"xq{g}") for g in range(2)]
                    wq = pp.tile([128, DCH * HD], dt.float16, tag="wq")
                    msk = pp.tile([128, RB * 256], dt.float16, tag="msk")
                    prj = pp.tile([128, 4 * D], dt.float16, tag="prj")
                    nc.sync.dma_start(out=xq[0][:], in_=XQ_d[:, 0])
                    nc.sync.dma_start(out=wq[:], in_=WQ_d[:])
                    nc.sync.dma_start(out=xq[1][:], in_=XQ_d[:, 1])
                    nc.sync.dma_start(out=msk[:], in_=MSK_d[:])
                    nc.sync.dma_start(out=prj[:], in_=PRJ_d[:])
                kps = [ps.tile([128, 512], dt.float32, tag=f"kps{h}",
                               name=f"kps{h}") for h in range(4)]
                vps = [ps.tile([128, 512], dt.float32, tag=f"vps{j}",
                               name=f"vps{j}") for j in range(4)]
                for c in range(DCH):
                    xt = xg[rt][:, 512 * c:512 * c + 512]
                    st = (c == 0)
                    sp = (c == DCH - 1)
                    for h in range(4):
                        nc.tensor.matmul(kps[h][:],
                                         wk[:, HD * c + 128 * h:HD * c + 128 * h + 128],
                                         xt, start=st, stop=sp)
                    for j in range(4):
                        nc.tensor.matmul(vps[j][:],
                                         xt[:, 128 * j:128 * j + 128],
                                         wv[:, HD * c:HD * c + HD],
                                         start=st, stop=sp)
                for j in range(4):
                    nc.vector.tensor_copy(V_sb[4 * rt + j][:], vps[j][:])
                for h in range(4):
                    nc.scalar.activation(
                        KT_sb[h][:, 512 * rt:512 * rt + 512], kps[h][:],
                        Act.Identity, bias=bias_sb["bK", h][:], scale=1.0)

            # ---- stage Q: Qc^T for this core's 1024 rows -------------------
            for rt in range(2):
                # reuse the V banks: the K banks then free right after the
                # last K copy, so attention's first S matmuls start sooner
                qps = [ps.tile([128, 512], dt.float32, tag=f"vps{h}",
                               name=f"qps{h}") for h in range(4)]
                for c in range(DCH):
                    xt = xq[rt][:, 512 * c:512 * c + 512]
                    for h in range(4):
                        nc.tensor.matmul(qps[h][:],
                                         wq[:, HD * c + 128 * h:HD * c + 128 * h + 128],
                                         xt, start=(c == 0), stop=(c == DCH - 1))
                for h in range(4):
                    nc.scalar.activation(
                        QT_sb[h][:, 512 * rt:512 * rt + 512], qps[h][:],
                        Act.Identity, bias=bias_sb["bQ", h][:], scale=1.0)

            # ---- stages A+O: attention + output projection, pipelined ------
            # Same PSUM pool, explicit per-bank tag cycling: each attention
            # tile waits only on the previous user of its specific bank, so
            # the first S matmuls start as soon as the last K copy frees a
            # K bank (while the Q stage still occupies the V banks).
            _cyc = {"sps": 0, "zps": 0, "etp": 0}
            _fam = {"sps": ["kps0", "kps1", "kps2"],
                    "zps": ["kps3", "vps3"],
                    "etp": ["vps0", "vps1", "vps2"]}

            def ps_tile(shape, dtype, fam, name):
                tags = _fam[fam]
                tag = tags[_cyc[fam] % len(tags)]
                _cyc[fam] += 1
                return ps.tile(shape, dtype, tag=tag, bufs=1, name=name)

            state = {}

            def stage_s(g):
                """S matmuls + mask + per-tile max for row block g."""
                ntile = KT_TILES[g]
                mpart = ap.tile([128, 8], dt.float32, tag="mpart", bufs=2,
                                name="mpart")
                s_tiles = []
                for kt in range(ntile):
                    sps = ps_tile([128, 256], dt.float32, "sps", "sps")
                    for h in range(4):
                        nc.tensor.matmul(
                            sps[:], QT_sb[h][:, 128 * g:128 * g + 128],
                            KT_sb[h][:, 256 * kt:256 * kt + 256],
                            start=(h == 0), stop=(h == 3))
                    ssb = ap.tile([128, 256], dt.float32, tag="ssb", bufs=17,
                                  name="ssb")
                    if kt == ntile - 1:
                        nc.vector.tensor_add(ssb[:], sps[:],
                                             msk[:, 256 * g:256 * g + 256])
                    else:
                        nc.vector.tensor_copy(ssb[:], sps[:])
                    nc.vector.reduce_max(mpart[:, kt:kt + 1], ssb[:], axis=Ax.X)
                    s_tiles.append(ssb)
                state[g] = (s_tiles, mpart)

            def stage_e(g):
                """negmax + exp + row sums + 1/(sum*sqrt(hs)) for block g."""
                ntile = KT_TILES[g]
                s_tiles, mpart = state[g]
                negm = ap.tile([128, 1], dt.float32, tag="negm", bufs=2,
                               name="negm")
                nc.vector.reduce_max(negm[:], mpart[:, 0:ntile], axis=Ax.X,
                                     negate=True)
                esum = ap.tile([128, 8], dt.float32, tag="esum", bufs=2,
                               name="esum")
                e_tiles = []
                for kt in range(ntile):
                    esb = ap.tile([128, 256], dt.float16, tag="esb", bufs=17,
                                  name="esb")
                    nc.scalar.activation(
                        esb[:], s_tiles[kt][:], Act.Exp,
                        bias=negm[:], scale=1.0,
                        accum_out=esum[:, kt:kt + 1])
                    e_tiles.append(esb)
                stot = ap.tile([128, 1], dt.float32, tag="stot", bufs=2,
                               name="stot")
                nc.vector.reduce_sum(stot[:], esum[:, 0:ntile], axis=Ax.X)
                stot2 = ap.tile([128, 1], dt.float32, tag="stot2", bufs=2,
                                name="stot2")
                nc.scalar.mul(stot2[:], stot[:], float(np.sqrt(128.0)))
                inv = ap.tile([128, 1], dt.float32, tag="inv", bufs=2,
                              name="inv")
                nc.vector.reciprocal(inv[:], stot2[:])
                state[g] = (e_tiles, inv)

            def stage_z_acc(g):
                """E^T transposes, Z accumulation, normalize (DVE)."""
                ntile = KT_TILES[g]
                e_tiles, inv = state.pop(g)
                zps = ps_tile([128, 512], dt.float32, "zps", "zps")
                nmm = 0
                for kt in range(ntile):
                    for j in range(2):
                        etp = ps_tile([128, 128], dt.float16, "etp", "etp")
                        nc.tensor.transpose(
                            etp[:], e_tiles[kt][:, 128 * j:128 * j + 128],
                            ident[:])
                        ets = ap.tile([128, 128], dt.float16, tag="ets",
                                      bufs=3, name="ets")
                        nc.vector.tensor_copy(ets[:], etp[:])
                        nc.tensor.matmul(
                            zps[:], ets[:], V_sb[2 * kt + j][:],
                            start=(nmm == 0), stop=(nmm == 2 * ntile - 1))
                        nmm += 1
                zn = ap.tile([128, 512], dt.float16, tag="zn", bufs=2,
                             name="zn")
                nc.vector.tensor_scalar_mul(zn[:], zps[:], inv[:])
                state[g, "zn"] = zn

            def stage_z_out(g, js):
                """Z^T transposes + ZT copies (+bV bias)."""
                zn = state[(g, "zn")]
                for j in js:
                    ztp = ps_tile([128, 128], dt.float16, "etp", "ztp")
                    nc.tensor.transpose(ztp[:], zn[:, 128 * j:128 * j + 128],
                                        ident[:])
                    nc.scalar.activation(
                        ZT_sb[j][:, 128 * g:128 * g + 128], ztp[:],
                        Act.Identity, bias=bias_sb["bV", j][:], scale=1.0)

            def stage_o(g):
                """output projection + per-chunk DMA for row block g."""
                osb = ap.tile([128, D], dt.float32, tag="osb", bufs=2,
                              name="osb")
                for dtile in range(4):
                    ops = ps_tile([128, 512], dt.float32, "sps", "ops")
                    for h in range(4):
                        nc.tensor.matmul(
                            ops[:], ZT_sb[h][:, 128 * g:128 * g + 128],
                            prj[:, D * h + 512 * dtile:D * h + 512 * dtile + 512],
                            start=(h == 0), stop=(h == 3))
                    nc.vector.tensor_copy(osb[:, 512 * dtile:512 * dtile + 512],
                                          ops[:])
                    nc.sync.dma_start(
                        out=OUT_d[128 * g:128 * g + 128,
                                  512 * dtile:512 * dtile + 512],
                        in_=osb[:, 512 * dtile:512 * dtile + 512])

            # software pipeline: PE stream per iteration is
            #   S(next) -> E^T/Z(cur) -> O(prev) -> Z^T(cur)
            # exp(cur) (scalar) hides behind S(next); the zn normalize
            # (DVE) hides behind O(prev).  Block order: first block reads
            # an early-finished Q group, big blocks prime the middle, the
            # last block's exposed Z+O tail is minimal.
            order = [2, 3, 7, 6, 5, 4, 1, 0]
            stage_s(order[0])
            stage_e(order[0])
            stage_s(order[1])
            stage_e(order[1])
            for i, g in enumerate(order):
                stage_z_acc(g)
                if i + 2 < RB:
                    stage_s(order[i + 2])
                    stage_e(order[i + 2])
                if i > 0:
                    stage_o(order[i - 1])
                stage_z_out(g, (0, 1, 2, 3))
                state.pop((g, "zn"))
            stage_o(order[-1])
            ps_cm.__exit__(None, None, None)

    nc.compile()
    return nc


def host_prep(x, WQ, bQ, WK, bK, WV, bV, proj):
    """Collapse weights, build fp16 DMA-friendly layouts, per-core maps."""
    x = np.asarray(x, dtype=np.float32)
    WQc = WQ.reshape(4, HD, D).sum(0)
    bQc = bQ.reshape(4, HD).sum(0)
    projc = proj.reshape(4, HD, D).sum(0)

    def wlayout(wt):  # [HD, D] -> W^T [D, HD] -> [128, DCH, HD] fp16
        t = np.ascontiguousarray(wt.T).reshape(DCH, 128, HD)
        return np.ascontiguousarray(t.transpose(1, 0, 2)).astype(np.float16)

    WKh = wlayout(WK)
    WVh = wlayout(WV)
    WQh = wlayout(WQc)
    # projc [HD, D] -> [4, 128, D] -> [128, 4, D]
    PRJ = np.ascontiguousarray(
        projc.reshape(4, 128, D).transpose(1, 0, 2)).astype(np.float16)
    bQc = np.ascontiguousarray(bQc.reshape(HD, 1)).astype(np.float32)
    bKc = np.ascontiguousarray(bK.reshape(HD, 1)).astype(np.float32)
    bVs = np.ascontiguousarray(
        (bV / np.sqrt(128.0)).reshape(HD, 1)).astype(np.float32)
    idt = np.eye(128, dtype=np.float16)

    in_maps = []
    for core in range(NCORES):
        b, q = divmod(core, 2)
        xT = x[b].T                                  # [D, T]
        # key-cols: [D, T] -> [DCH, 128, 4, 512] -> [128, 4, DCH, 512]
        XKV = np.ascontiguousarray(
            xT.reshape(DCH, 128, 4, 512).transpose(1, 2, 0, 3)
        ).astype(np.float16)
        rows = np.concatenate(
            [np.arange(256 * g + 128 * q, 256 * g + 128 * q + 128)
             for g in range(RB)])
        xTq = xT[:, rows]                            # [D, 1024]
        XQ = np.ascontiguousarray(
            xTq.reshape(DCH, 128, 2, 512).transpose(1, 2, 0, 3)
        ).astype(np.float16)
        msk = np.zeros((RB, 128, 256), dtype=np.float16)
        for g in range(RB):
            ntile = KT_TILES[g]
            base = 256 * (ntile - 1)                 # keys covered by last tile
            key = base + np.arange(256)[None, :]
            row = (256 * g + 128 * q + np.arange(128))[:, None]
            msk[g] = np.where(key <= row, np.float16(0.0), np.float16(NEG))
        MSK = np.ascontiguousarray(msk.transpose(1, 0, 2))   # [128, RB, 256]
        in_maps.append({
            "XKV": XKV, "XQ": XQ, "WK": WKh, "WV": WVh, "WQ": WQh,
            "PRJ": PRJ, "bK": bKc, "bQ": bQc, "bVs": bVs,
            "MSK": MSK, "IDT": idt,
        })
    return in_maps


def assemble(results):
    """Gather per-core [1024, D] outputs into [B, T, D]."""
    y = np.empty((B, T, D), dtype=np.float32)
    for core in range(NCORES):
        b, q = divmod(core, 2)
        o = results[core]["out"]
        for g in range(RB):
            y[b, 256 * g + 128 * q:256 * g + 128 * q + 128] = \
                o[128 * g:128 * g + 128]
    return y


_NC_CACHE = None


def kernel(x, WQ, bQ, WK, bK, WV, bV, proj):
    global _NC_CACHE
    in_maps = host_prep(np.asarray(x), np.asarray(WQ), np.asarray(bQ),
                        np.asarray(WK), np.asarray(bK), np.asarray(WV),
                        np.asarray(bV), np.asarray(proj))
    if _NC_CACHE is None:
        _NC_CACHE = build_kernel()
    res = run_bass_kernel_spmd(_NC_CACHE, in_maps, list(range(NCORES)))
    return assemble(res.results)


# revision 56
# speedup vs baseline: 1.1744x; 1.1744x over previous
"""GroupedQueryAttention Trainium2 Bass kernel (8 NeuronCores, SPMD).

Reference quirk exploited: K/V are tiled R=4x along the group axis and
attention runs over the full concatenated 2048-dim. Mathematically this
collapses:
  scores = Q . tile(K)  ==  (sum of Q's four 512-chunks) . K      (512-dim)
  Z      = attn . tile(V)  -> tiled copies of  attn . V           (512-dim)
  out    = Z @ proj     ==  (attn . V) @ (sum of proj's 4 row-blocks)
So the whole module reduces to a single 512-dim attention:
  Qc = x @ WQc.T + bQc   (WQc = sum of WQ row-blocks)
  K  = x @ WK.T + bK ; V = x @ WV.T        (bV folded: see below)
  S  = Qc K^T (causal), softmax, /sqrt(128)
  y  = (softmax(S)/sqrt(128) V) @ projc    (projc = sum of proj row-blocks)
Because softmax rows sum to exactly 1, attn rows sum to 1/sqrt(128), so
attn @ (V + 1 bV^T) = attn @ V + bV/sqrt(128): the V bias becomes a
constant per-channel offset applied on the Z^T PSUM->SBUF copy instead
of rank-1 matmuls.

Sharding: 8 cores = 4 batches x 2 (interleaved 128-row blocks). Core with
pairpos q of batch b owns rows {256g+128q .. 256g+128q+127, g=0..7}. Causal
key extent per block rounds up to 256-multiples, which makes the tile counts
[1,2,3,4,5,6,7,8] identical on both cores of a pair -> a single SPMD program.
Per-core row selection / causal masks are host-prepared input tensors.

All matmul inputs are fp16 (full-speed PE mode, fp32 PSUM accumulation;
softmax statistics in fp32) — halves DMA traffic and LDWEIGHTS bandwidth
vs f32r at identical PE cycles/row. Host pre-arranges every tensor so
each device DMA is a single contiguous-per-partition descriptor; early
DMAs are chunk-staggered so the first matmul issues ~9us in.
Attention row-blocks are software-pipelined two deep; per iteration the
PE stream is  E^T/Z(cur) -> S(cur+2) -> O(prev) -> Z^T(cur)  so exp()
(scalar) and the zn normalize (DVE) always have PE work to hide behind,
and per-512-chunk output DMAs overlap compute. PSUM is managed as
explicit per-bank tag rings (S/O: 3, Z: 2, transposes: 3) so each tile
waits only on its own bank's previous user, never on a pool barrier.
"""

import numpy as np

import concourse.bacc as bacc
import concourse.mybir as mybir
from concourse.tile import TileContext
from concourse.bass_utils import run_bass_kernel_spmd

B, T, D = 4, 2048, 2048
HD = 512                 # collapsed head dim
NCORES = 8
RB = 8                   # 128-row blocks per core
KT_TILES = [1, 2, 3, 4, 5, 6, 7, 8]   # causal 256-wide key tiles per block
DCH = D // 128           # 16 contraction chunks
dt = mybir.dt
NEG = -60000.0           # fp16-safe mask value


def build_kernel():
    nc = bacc.Bacc(None, target_bir_lowering=False)

    # host-prepared layouts (see host_prep):
    #   XKV [128, 4, 16, 512] : x^T key-cols   (partition, key-group, c, key)
    #   XQ  [128, 2, 16, 512] : x^T query-cols (partition, row-group, c, row)
    #   WK/WV/WQ [128, 16, 512]: W^T           (partition, c, hd)
    #   PRJ [128, 4, 2048]    : projc          (partition, h, d)
    #   MSK [128, 8, 256]     : causal mask    (partition=row, block, key)
    XKV_d = nc.dram_tensor("XKV", [128, 4, DCH, 512], dt.float16, kind="ExternalInput")
    XQ_d = nc.dram_tensor("XQ", [128, 2, DCH, 512], dt.float16, kind="ExternalInput")
    WK_d = nc.dram_tensor("WK", [128, DCH, HD], dt.float16, kind="ExternalInput")
    WV_d = nc.dram_tensor("WV", [128, DCH, HD], dt.float16, kind="ExternalInput")
    WQ_d = nc.dram_tensor("WQ", [128, DCH, HD], dt.float16, kind="ExternalInput")
    PRJ_d = nc.dram_tensor("PRJ", [128, 4, D], dt.float16, kind="ExternalInput")
    bK_d = nc.dram_tensor("bK", [HD, 1], dt.float32, kind="ExternalInput")
    bQ_d = nc.dram_tensor("bQ", [HD, 1], dt.float32, kind="ExternalInput")
    bVs_d = nc.dram_tensor("bVs", [HD, 1], dt.float32, kind="ExternalInput")
    MSK_d = nc.dram_tensor("MSK", [128, RB, 256], dt.float16, kind="ExternalInput")
    IDT_d = nc.dram_tensor("IDT", [128, 128], dt.float16, kind="ExternalInput")
    OUT_d = nc.dram_tensor("out", [1024, D], dt.float32, kind="ExternalOutput")

    Act = mybir.ActivationFunctionType
    Ax = mybir.AxisListType

    with TileContext(nc) as tc:
        with tc.tile_pool(name="persist", bufs=1) as pp, \
             tc.tile_pool(name="xstream", bufs=2) as xp, \
             tc.tile_pool(name="scratch", bufs=1) as ap:
            # ---- persistent tiles ------------------------------------------
            wk = pp.tile([128, DCH * HD], dt.float16, tag="wk")
            xg = [xp.tile([128, DCH * 512], dt.float16, tag="xg", name=f"xg{rt}")
                  for rt in range(4)]
            # first-needed-first DMA order: single chunks unblock matmul 0 fast
            wv = pp.tile([128, DCH * HD], dt.float16, tag="wv")
            # stagger arrivals to match per-chunk consumption at the PE
            for lo, hi in ((0, 1), (1, 2), (2, 3), (3, 4), (4, 8), (8, 12),
                           (12, 16)):
                nc.sync.dma_start(out=wk[:, HD * lo:HD * hi], in_=WK_d[:, lo:hi])
                nc.sync.dma_start(out=xg[0][:, 512 * lo:512 * hi],
                                  in_=XKV_d[:, 0, lo:hi])
                nc.sync.dma_start(out=wv[:, HD * lo:HD * hi], in_=WV_d[:, lo:hi])
            KT_sb = [pp.tile([128, T], dt.float16, tag=f"KT{h}", name=f"KT{h}")
                     for h in range(4)]
            V_sb = [pp.tile([128, HD], dt.float16, tag=f"V{k}", name=f"V{k}")
                    for k in range(16)]
            QT_sb = [pp.tile([128, 1024], dt.float16, tag=f"QT{h}", name=f"QT{h}")
                     for h in range(4)]
            ZT_sb = [pp.tile([128, 1024], dt.float16, tag=f"ZT{h}", name=f"ZT{h}")
                     for h in range(4)]
            ident = pp.tile([128, 128], dt.float16, tag="ident")
            nc.sync.dma_start(out=ident[:], in_=IDT_d[:])
            bias_sb = {}
            for nm, src in (("bK", bK_d), ("bQ", bQ_d), ("bV", bVs_d)):
                for h in range(4):
                    t = pp.tile([128, 1], dt.float32, tag=f"{nm}{h}", name=f"{nm}{h}")
                    nc.sync.dma_start(out=t[:], in_=src[128 * h:128 * h + 128, :])
                    bias_sb[nm, h] = t

            # ---- stage P: K/V projections ----------------------------------
            # Per 512-key group: 4 PSUM banks accumulate K^T hd-tiles
            # ([hd, keys]) and 4 banks accumulate V key-blocks in [keys, hd]
            # layout (x chunk as lhsT).  K bias lands in the PSUM->SBUF copy.
            ps_cm = tc.tile_pool(name="psP", bufs=1, space="PSUM")
            ps = ps_cm.__enter__()
            for rt in range(4):
                if rt + 1 < 4:   # prefetch next x key-group
                    nc.sync.dma_start(out=xg[rt + 1][:], in_=XKV_d[:, rt + 1])
                if rt == 2:      # then queue the rest of the streamed inputs
                    xq = [xp.tile([128, DCH * 512], dt.float16, tag="xg",
                                  name=f"xq{g}") for g in range(2)]
                    wq = pp.tile([128, DCH * HD], dt.float16, tag="wq")
                    msk = pp.tile([128, RB * 256], dt.float16, tag="msk")
                    prj = pp.tile([128, 4 * D], dt.float16, tag="prj")
                    nc.sync.dma_start(out=xq[0][:], in_=XQ_d[:, 0])
                    nc.sync.dma_start(out=wq[:], in_=WQ_d[:])
                    nc.sync.dma_start(out=xq[1][:], in_=XQ_d[:, 1])
                    nc.sync.dma_start(out=msk[:], in_=MSK_d[:])
                    nc.sync.dma_start(out=prj[:], in_=PRJ_d[:])
                kps = [ps.tile([128, 512], dt.float32, tag=f"kps{h}",
                               name=f"kps{h}") for h in range(4)]
                vps = [ps.tile([128, 512], dt.float32, tag=f"vps{j}",
                               name=f"vps{j}") for j in range(4)]
                for c in range(DCH):
                    xt = xg[rt][:, 512 * c:512 * c + 512]
                    st = (c == 0)
                    sp = (c == DCH - 1)
                    for h in range(4):
                        nc.tensor.matmul(kps[h][:],
                                         wk[:, HD * c + 128 * h:HD * c + 128 * h + 128],
                                         xt, start=st, stop=sp)
                    for j in range(4):
                        nc.tensor.matmul(vps[j][:],
                                         xt[:, 128 * j:128 * j + 128],
                                         wv[:, HD * c:HD * c + HD],
                                         start=st, stop=sp)
                for j in range(4):
                    nc.vector.tensor_copy(V_sb[4 * rt + j][:], vps[j][:])
                for h in range(4):
                    nc.scalar.activation(
                        KT_sb[h][:, 512 * rt:512 * rt + 512], kps[h][:],
                        Act.Identity, bias=bias_sb["bK", h][:], scale=1.0)

            # ---- stage Q: Qc^T for this core's 1024 rows -------------------
            for rt in range(2):
                # reuse the V banks: the K banks then free right after the
                # last K copy, so attention's first S matmuls start sooner
                qps = [ps.tile([128, 512], dt.float32, tag=f"vps{h}",
                               name=f"qps{h}") for h in range(4)]
                for c in range(DCH):
                    xt = xq[rt][:, 512 * c:512 * c + 512]
                    for h in range(4):
                        nc.tensor.matmul(qps[h][:],
                                         wq[:, HD * c + 128 * h:HD * c + 128 * h + 128],
                                         xt, start=(c == 0), stop=(c == DCH - 1))
                for h in range(4):
                    nc.scalar.activation(
                        QT_sb[h][:, 512 * rt:512 * rt + 512], qps[h][:],
                        Act.Identity, bias=bias_sb["bQ", h][:], scale=1.0)

            # ---- stages A+O: attention + output projection, pipelined ------
            # Same PSUM pool, explicit per-bank tag cycling: each attention
            # tile waits only on the previous user of its specific bank, so
            # the first S matmuls start as soon as the last K copy frees a
            # K bank (while the Q stage still occupies the V banks).
            _cyc = {"sps": 0, "zps": 0, "etp": 0}
            _fam = {"sps": ["kps0", "kps1", "kps2", "vps2"],
                    "zps": ["kps3", "vps3"],
                    "etp": ["vps0", "vps1"]}

            def ps_tile(shape, dtype, fam, name):
                tags = _fam[fam]
                tag = tags[_cyc[fam] % len(tags)]
                _cyc[fam] += 1
                return ps.tile(shape, dtype, tag=tag, bufs=1, name=name)

            state = {}

            def stage_s(g):
                """S matmuls + mask + per-tile max for row block g."""
                ntile = KT_TILES[g]
                mpart = ap.tile([128, 8], dt.float32, tag="mpart", bufs=2,
                                name="mpart")
                s_tiles = []
                for kt in range(ntile):
                    sps = ps_tile([128, 256], dt.float32, "sps", "sps")
                    for h in range(4):
                        nc.tensor.matmul(
                            sps[:], QT_sb[h][:, 128 * g:128 * g + 128],
                            KT_sb[h][:, 256 * kt:256 * kt + 256],
                            start=(h == 0), stop=(h == 3))
                    ssb = ap.tile([128, 256], dt.float32, tag="ssb", bufs=17,
                                  name="ssb")
                    if kt == ntile - 1:
                        nc.vector.tensor_add(ssb[:], sps[:],
                                             msk[:, 256 * g:256 * g + 256])
                    else:
                        nc.vector.tensor_copy(ssb[:], sps[:])
                    nc.vector.reduce_max(mpart[:, kt:kt + 1], ssb[:], axis=Ax.X)
                    s_tiles.append(ssb)
                state[g] = (s_tiles, mpart)

            def stage_e(g):
                """negmax + exp + row sums + 1/(sum*sqrt(hs)) for block g."""
                ntile = KT_TILES[g]
                s_tiles, mpart = state[g]
                negm = ap.tile([128, 1], dt.float32, tag="negm", bufs=2,
                               name="negm")
                nc.vector.reduce_max(negm[:], mpart[:, 0:ntile], axis=Ax.X,
                                     negate=True)
                esum = ap.tile([128, 8], dt.float32, tag="esum", bufs=2,
                               name="esum")
                e_tiles = []
                for kt in range(ntile):
                    esb = ap.tile([128, 256], dt.float16, tag="esb", bufs=17,
                                  name="esb")
                    nc.scalar.activation(
                        esb[:], s_tiles[kt][:], Act.Exp,
                        bias=negm[:], scale=1.0,
                        accum_out=esum[:, kt:kt + 1])
                    e_tiles.append(esb)
                stot = ap.tile([128, 1], dt.float32, tag="stot", bufs=2,
                               name="stot")
                nc.vector.reduce_sum(stot[:], esum[:, 0:ntile], axis=Ax.X)
                stot2 = ap.tile([128, 1], dt.float32, tag="stot2", bufs=2,
                                name="stot2")
                nc.scalar.mul(stot2[:], stot[:], float(np.sqrt(128.0)))
                inv = ap.tile([128, 1], dt.float32, tag="inv", bufs=2,
                              name="inv")
                nc.vector.reciprocal(inv[:], stot2[:])
                state[g] = (e_tiles, inv)

            def stage_z_acc(g):
                """E^T transposes, Z accumulation, normalize (DVE)."""
                ntile = KT_TILES[g]
                e_tiles, inv = state.pop(g)
                zps = ps_tile([128, 512], dt.float32, "zps", "zps")
                nmm = 0
                for kt in range(ntile):
                    for j in range(2):
                        etp = ps_tile([128, 128], dt.float16, "etp", "etp")
                        nc.tensor.transpose(
                            etp[:], e_tiles[kt][:, 128 * j:128 * j + 128],
                            ident[:])
                        ets = ap.tile([128, 128], dt.float16, tag="ets",
                                      bufs=3, name="ets")
                        nc.vector.tensor_copy(ets[:], etp[:])
                        nc.tensor.matmul(
                            zps[:], ets[:], V_sb[2 * kt + j][:],
                            start=(nmm == 0), stop=(nmm == 2 * ntile - 1))
                        nmm += 1
                zn = ap.tile([128, 512], dt.float16, tag="zn", bufs=2,
                             name="zn")
                nc.vector.tensor_scalar_mul(zn[:], zps[:], inv[:])
                state[g, "zn"] = zn

            def stage_z_out(g, js):
                """Z^T transposes + ZT copies (+bV bias)."""
                zn = state[(g, "zn")]
                for j in js:
                    ztp = ps_tile([128, 128], dt.float16, "etp", "ztp")
                    nc.tensor.transpose(ztp[:], zn[:, 128 * j:128 * j + 128],
                                        ident[:])
                    nc.scalar.activation(
                        ZT_sb[j][:, 128 * g:128 * g + 128], ztp[:],
                        Act.Identity, bias=bias_sb["bV", j][:], scale=1.0)

            def stage_o(g):
                """output projection + per-chunk DMA for row block g."""
                osb = ap.tile([128, D], dt.float32, tag="osb", bufs=2,
                              name="osb")
                for dtile in range(4):
                    ops = ps_tile([128, 512], dt.float32, "sps", "ops")
                    for h in range(4):
                        nc.tensor.matmul(
                            ops[:], ZT_sb[h][:, 128 * g:128 * g + 128],
                            prj[:, D * h + 512 * dtile:D * h + 512 * dtile + 512],
                            start=(h == 0), stop=(h == 3))
                    nc.vector.tensor_copy(osb[:, 512 * dtile:512 * dtile + 512],
                                          ops[:])
                    nc.sync.dma_start(
                        out=OUT_d[128 * g:128 * g + 128,
                                  512 * dtile:512 * dtile + 512],
                        in_=osb[:, 512 * dtile:512 * dtile + 512])

            # software pipeline: PE stream per iteration is
            #   S(next) -> E^T/Z(cur) -> O(prev) -> Z^T(cur)
            # exp(cur) (scalar) hides behind S(next); the zn normalize
            # (DVE) hides behind O(prev).  Block order: first block reads
            # an early-finished Q group, big blocks prime the middle, the
            # last block's exposed Z+O tail is minimal.
            order = [2, 3, 7, 6, 5, 4, 1, 0]
            stage_s(order[0])
            stage_e(order[0])
            stage_s(order[1])
            stage_e(order[1])
            for i, g in enumerate(order):
                stage_z_acc(g)
                if i + 2 < RB:
                    stage_s(order[i + 2])
                    stage_e(order[i + 2])
                if i > 0:
                    stage_o(order[i - 1])
                stage_z_out(g, (0, 1, 2, 3))
                state.pop((g, "zn"))
            stage_o(order[-1])
            ps_cm.__exit__(None, None, None)

    nc.compile()
    return nc


def host_prep(x, WQ, bQ, WK, bK, WV, bV, proj):
    """Collapse weights, build fp16 DMA-friendly layouts, per-core maps."""
    x = np.asarray(x, dtype=np.float32)
    WQc = WQ.reshape(4, HD, D).sum(0)
    bQc = bQ.reshape(4, HD).sum(0)
    projc = proj.reshape(4, HD, D).sum(0)

    def wlayout(wt):  # [HD, D] -> W^T [D, HD] -> [128, DCH, HD] fp16
        t = np.ascontiguousarray(wt.T).reshape(DCH, 128, HD)
        return np.ascontiguousarray(t.transpose(1, 0, 2)).astype(np.float16)

    WKh = wlayout(WK)
    WVh = wlayout(WV)
    WQh = wlayout(WQc)
    # projc [HD, D] -> [4, 128, D] -> [128, 4, D]
    PRJ = np.ascontiguousarray(
        projc.reshape(4, 128, D).transpose(1, 0, 2)).astype(np.float16)
    bQc = np.ascontiguousarray(bQc.reshape(HD, 1)).astype(np.float32)
    bKc = np.ascontiguousarray(bK.reshape(HD, 1)).astype(np.float32)
    bVs = np.ascontiguousarray(
        (bV / np.sqrt(128.0)).reshape(HD, 1)).astype(np.float32)
    idt = np.eye(128, dtype=np.float16)

    in_maps = []
    for core in range(NCORES):
        b, q = divmod(core, 2)
        xT = x[b].T                                  # [D, T]
        # key-cols: [D, T] -> [DCH, 128, 4, 512] -> [128, 4, DCH, 512]
        XKV = np.ascontiguousarray(
            xT.reshape(DCH, 128, 4, 512).transpose(1, 2, 0, 3)
        ).astype(np.float16)
        rows = np.concatenate(
            [np.arange(256 * g + 128 * q, 256 * g + 128 * q + 128)
             for g in range(RB)])
        xTq = xT[:, rows]                            # [D, 1024]
        XQ = np.ascontiguousarray(
            xTq.reshape(DCH, 128, 2, 512).transpose(1, 2, 0, 3)
        ).astype(np.float16)
        msk = np.zeros((RB, 128, 256), dtype=np.float16)
        for g in range(RB):
            ntile = KT_TILES[g]
            base = 256 * (ntile - 1)                 # keys covered by last tile
            key = base + np.arange(256)[None, :]
            row = (256 * g + 128 * q + np.arange(128))[:, None]
            msk[g] = np.where(key <= row, np.float16(0.0), np.float16(NEG))
        MSK = np.ascontiguousarray(msk.transpose(1, 0, 2))   # [128, RB, 256]
        in_maps.append({
            "XKV": XKV, "XQ": XQ, "WK": WKh, "WV": WVh, "WQ": WQh,
            "PRJ": PRJ, "bK": bKc, "bQ": bQc, "bVs": bVs,
            "MSK": MSK, "IDT": idt,
        })
    return in_maps


def assemble(results):
    """Gather per-core [1024, D] outputs into [B, T, D]."""
    y = np.empty((B, T, D), dtype=np.float32)
    for core in range(NCORES):
        b, q = divmod(core, 2)
        o = results[core]["out"]
        for g in range(RB):
            y[b, 256 * g + 128 * q:256 * g + 128 * q + 128] = \
                o[128 * g:128 * g + 128]
    return y


_NC_CACHE = None


def kernel(x, WQ, bQ, WK, bK, WV, bV, proj):
    global _NC_CACHE
    in_maps = host_prep(np.asarray(x), np.asarray(WQ), np.asarray(bQ),
                        np.asarray(WK), np.asarray(bK), np.asarray(WV),
                        np.asarray(bV), np.asarray(proj))
    if _NC_CACHE is None:
        _NC_CACHE = build_kernel()
    res = run_bass_kernel_spmd(_NC_CACHE, in_maps, list(range(NCORES)))
    return assemble(res.results)
